# revision 15
# baseline (speedup 1.0000x reference)
"""Trainium2 Bass kernel for nn_DynamicShortConvolution.

Reference computation (per token t, channel d):
    h    = silu(x @ w1)                       # [T, H]
    flat = h @ w2 + b2                        # [T, D*W]
    k    = flat.reshape(T, D, W)
    out[t, d] = silu(sum_w k[t, d, w] * x[t - (W-1) + w, d])

Sharding: 8 cores, each one (batch, half-of-T) shard of 2048 tokens plus a
3-token left halo.  All per-core tensors are laid out TRANSPOSED ([D, T],
channels on SBUF partitions) so the conv's token shift is a free-dim offset.

v5: the steady state (~2.7us per 512-token x 128-channel iteration) is at
its engine-balance optimum: DVE (2 stt + 2 adds) ~= GPSIMD (2 multiplies)
~= 2.7us, with ACT (2 psum pulls + silu) and PE (8+ matmuls) just under.
The remaining ~45us were lead-in and chunk-boundary overhead:
  - x is stored CHUNK-MAJOR in DRAM ([128, chunk, dt, 516], halo columns
    duplicated per chunk) so one chunk moves with 128 x 16.5KB descriptors
    instead of 2048 x 1KB - the first chunk lands ~6us earlier and DMA
    queues stay quiet during compute.
  - w1 is stored [128, hc, dt, 128] and sent as two DMAs; chunk 0 of x is
    sent as four quarter-DMAs so mm1's contraction steps chase arrival.
  - ~22 warmup matmuls on a memset scratch tile run while the input DMA
    is in flight, so mm1 executes at full clock (2.4GHz) instead of
    ramping from the 0.65GHz cold p-state.
  - mm1 for chunk c+1 is spread over iterations 4..11 of chunk c (silu at
    12) instead of 8..15 (silu at 15), removing the chunk-boundary stall.
"""

import numpy as np

# Problem constants (hardcoded per harness contract).
B, T, D, H, W = 4, 4096, 2048, 256, 4
HALO = W - 1
N_CORES = 8
TOK = (B * T) // N_CORES  # tokens per core = 2048
TCH = 512                 # token chunk
XSTR = TCH + HALO + 1     # per-(chunk,dtile) x columns (pad to even)


def _build_nc(tok, d, h, sim_safe=False):
    """Build the single-core Bass/Tile program."""
    import concourse.bass as bass
    import concourse.bacc as bacc
    import concourse.mybir as mybir
    import concourse.tile as tile

    f32 = mybir.dt.float32
    bf16 = mybir.dt.bfloat16
    AF = mybir.ActivationFunctionType
    ALU = mybir.AluOpType
    # CoreSim has no Silu; Sigmoid exercises the identical dataflow
    AF_ACT = AF.Sigmoid if sim_safe else AF.Silu

    n_dt = d // 128        # d tiles = 16
    n_hc = h // 128        # h tiles = 2
    n_tc = tok // TCH      # token chunks = 4

    nc = bacc.Bacc()

    # DRAM I/O. x chunk-major with duplicated halo: big contiguous runs.
    xT = nc.declare_dram_parameter("xT", [128, n_tc, n_dt, XSTR], bf16,
                                   isOutput=False)
    # w1r[p, hc, dt, hl] = w1[dt*128+p, hc*128+hl]
    w1 = nc.declare_dram_parameter("w1", [128, n_hc, n_dt, 128], bf16,
                                   isOutput=False)
    # w2r[hl, hc, dt, w, dl] = w2[hc*128+hl, ((dt*128+dl))*W + w]
    w2r = nc.declare_dram_parameter("w2r", [128, n_hc, n_dt, W, 128], bf16,
                                    isOutput=False)
    # b2r[p, dt*W + w] = b2[(dt*128+p)*W + w]
    b2r = nc.declare_dram_parameter("b2r", [128, n_dt * W], f32, isOutput=False)
    # outT[p, (c*n_dt + dt)*TCH + j] = out token c*TCH+j, channel dt*128+p
    outT = nc.declare_dram_parameter("outT", [128, n_tc * n_dt * TCH], bf16,
                                     isOutput=True)

    def strided_ap(base, pairs):
        """Rebuild an AP with explicit free-dim [stride, count] pairs."""
        new = [list(base.ap[0])] + [list(p) for p in pairs]
        return type(base)(base.tensor, base.offset, new)

    with tile.TileContext(nc) as tc:
        with (
            tc.tile_pool(name="resident", bufs=1) as rpool,
            tc.tile_pool(name="work", bufs=4) as wpool,
            tc.tile_pool(name="outstage", bufs=3) as opool,
            tc.tile_pool(name="psum2", bufs=6, space="PSUM") as ppool,
            tc.tile_pool(name="psumH", bufs=2, space="PSUM") as hpool,
        ):
            # ---- resident tiles ----
            xT_sb = rpool.tile([128, n_tc, n_dt, XSTR], bf16, tag="xT")
            w1_sb = rpool.tile([128, n_hc, n_dt, 128], bf16, tag="w1")
            w2_sb = rpool.tile([128, n_hc, n_dt, W, 128], bf16, tag="w2")
            b2_sb = rpool.tile([128, n_dt * W], f32, tag="b2")
            # hT chunk-major: [hc0 512 | hc1 512] per chunk
            hT_sb = rpool.tile([128, n_tc * 2 * TCH], bf16, tag="hT")
            scr = rpool.tile([128, TCH], bf16, tag="scr")   # warmup operand
            scr2 = rpool.tile([128, 64], bf16, tag="scr2")  # warmup drain

            # ---- PE warmup: ~22 dummy matmuls while input DMA flies ----
            nc.gpsimd.memset(scr[:], 0)
            for ww in range(11):
                wps = ppool.tile([128, TCH], f32, tag="ps", name="warm")
                nc.tensor.matmul(wps[:], scr[:, :128], scr[:],
                                 start=True, stop=False)
                nc.tensor.matmul(wps[:], scr[:, :128], scr[:],
                                 start=False, stop=True)
                nc.scalar.activation(scr2[:], wps[:, :64], AF.Identity)

            # ---- input DMA, lead-in ordered ----
            for hc in range(n_hc):
                nc.sync.dma_start(w1_sb[:, hc], w1[:, hc])
            for dt in range(0, n_dt, 4):  # chunk 0 in quarters
                nc.sync.dma_start(xT_sb[:, 0, dt:dt + 4], xT[:, 0, dt:dt + 4])
            for hc in range(n_hc):
                nc.sync.dma_start(w2_sb[:, hc, 0:4], w2r[:, hc, 0:4])
            nc.sync.dma_start(b2_sb[:], b2r[:])
            nc.sync.dma_start(xT_sb[:, 1], xT[:, 1])
            for hc in range(n_hc):
                nc.sync.dma_start(w2_sb[:, hc, 4:8], w2r[:, hc, 4:8])
            # x2, x3, w2[8:16] are dispatched from inside the loop so their
            # SBUF-write bursts don't collide with the first iterations
            # (input DMA traffic measured as the cause of early stalls)

            def x_slice(c, dt, col, n):
                # col 0 = token c*TCH - HALO (halo duplicated per chunk)
                return xT_sb[:, c, dt, col: col + n]

            def b2s(dt, w):
                return b2_sb[:, dt * W + w: dt * W + w + 1]

            def hslice(c, hc):
                return hT_sb[:, c * 1024 + hc * TCH: c * 1024 + (hc + 1) * TCH]

            def mm1_mms(c, hps, q):
                # contraction tile q of mm1 for chunk c
                for hc in range(n_hc):
                    nc.tensor.matmul(
                        hps[hc][:],
                        w1_sb[:, hc, q],
                        x_slice(c, q, HALO, TCH),
                        start=(q == 0), stop=(q == n_dt - 1),
                    )

            def mm1_silu(c, hps):
                for hc in range(n_hc):
                    nc.scalar.activation(hslice(c, hc), hps[hc][:], AF_ACT)

            def hps_alloc():
                return [hpool.tile([128, TCH], f32, tag="hps", name=f"hps{hc}")
                        for hc in range(n_hc)]

            # ---- chunk 0 mm1 up front (chases the quarter DMAs) ----
            hps = hps_alloc()
            for q in range(n_dt):
                mm1_mms(0, hps, q)
            mm1_silu(0, hps)

            sw_q = []    # (mbuf, finbuf, half): s-wide + fin, 1-iter skew
            silu_q = []  # (finbuf, git): completed pairs awaiting silu

            def emit_sw():
                # s-wide add on GPSIMD (contiguous operands), fin on DVE
                while sw_q:
                    mb, fb, half = sw_q.pop(0)
                    sb = wpool.tile([128, 1024], bf16, tag="s")
                    nc.gpsimd.tensor_tensor(
                        sb[:], mb[:, :1024], mb[:, 1024:], op=ALU.add)
                    nc.vector.tensor_tensor(
                        fb[:, half * TCH:(half + 1) * TCH],
                        sb[:, :TCH], sb[:, TCH:], op=ALU.add)

            otbuf = [None]

            def emit_silu(drain=False):
                # one [128,4096] output DMA per four silu pairs
                while silu_q:
                    fb, git = silu_q.pop(0)
                    part = (git % 8) // 2
                    if part == 0:
                        otbuf[0] = opool.tile([128, 4096], bf16, tag="ot",
                                              name="ot")
                    ot = otbuf[0]
                    nc.scalar.activation(
                        ot[:, part * 1024:(part + 1) * 1024], fb[:], AF_ACT)
                    if part == 3 or drain:
                        nc.sync.dma_start(
                            outT[:, (git - 2 * part) * TCH:
                                 git * TCH + 1024], ot[:, :(part + 1) * 1024])

            finbuf = None
            for c in range(n_tc):
                hps_next = hps_alloc() if c + 1 < n_tc else None
                for dt in range(n_dt):
                    it = c * n_dt + dt
                    # deferred input DMA dispatches (Sync issues in order),
                    # split small so each SBUF-write burst is short
                    if it in (3, 6):
                        a = 8 if it == 3 else 12
                        for hc in range(n_hc):
                            nc.sync.dma_start(w2_sb[:, hc, a:a + 4],
                                              w2r[:, hc, a:a + 4])
                    elif 10 <= it <= 16 and it % 2 == 0:
                        q4 = 2 * (it - 10)  # 0,4,8,12
                        nc.sync.dma_start(xT_sb[:, 2, q4:q4 + 4],
                                          xT[:, 2, q4:q4 + 4])
                    elif 22 <= it <= 28 and it % 2 == 0:
                        q4 = 2 * (it - 22)
                        nc.sync.dma_start(xT_sb[:, 3, q4:q4 + 4],
                                          xT[:, 3, q4:q4 + 4])
                    # PE: per-tap psum tiles, ACT-feeding taps (0,2) first
                    p = [None] * W
                    for w in (0, 2, 1, 3):
                        pt = ppool.tile([128, TCH], f32, tag="ps")
                        p[w] = pt
                        for hc in range(n_hc):
                            nc.tensor.matmul(
                                pt[:], w2_sb[:, hc, dt, w], hslice(c, hc),
                                start=(hc == 0), stop=(hc == n_hc - 1),
                            )
                    # PE: two contraction tiles of mm1(c+1), spread over
                    # iters 4..11 (x for c+1 lands early with chunk-major
                    # DMA); silu at iter 12 so hT(c+1) is ready with slack.
                    # Chunk 0 spreads over 6..13 instead - its early iters
                    # are still filling the psum pipeline.
                    d0 = 6 if c == 0 else 4
                    if hps_next is not None and d0 <= dt < d0 + 8:
                        q = 2 * (dt - d0)
                        mm1_mms(c + 1, hps_next, q)
                        mm1_mms(c + 1, hps_next, q + 1)
                    # ACT: pull taps 0,2 to bf16 with the b2 bias fused
                    t02 = wpool.tile([128, 1024], bf16, tag="t02")
                    nc.scalar.activation(t02[:, :TCH], p[0][:],
                                         AF.Identity, bias=b2s(dt, 0))
                    nc.scalar.activation(t02[:, TCH:], p[2][:],
                                         AF.Identity, bias=b2s(dt, 2))
                    # DVE: both pulled-tap multiplies as ONE strided 2x op
                    #   mbuf[0:512]     = t02[0:512]    * x[j0+0 : j0+512]
                    #   mbuf[1024:1536] = t02[512:1024] * x[j0+2 : j0+514]
                    mbuf = wpool.tile([128, 2048], bf16, tag="m")
                    in0 = strided_ap(t02[:, 0:TCH], [[TCH, 2], [1, TCH]])
                    in1 = strided_ap(x_slice(c, dt, 0, TCH),
                                     [[2, 2], [1, TCH]])
                    out0 = strided_ap(mbuf[:, 0:TCH], [[1024, 2], [1, TCH]])
                    nc.vector.tensor_tensor(out0, in0, in1, op=ALU.mult)
                    # DVE: f32 stt (bias + x-mult) for taps 1,3
                    nc.vector.scalar_tensor_tensor(
                        mbuf[:, TCH:1024], p[1][:], b2s(dt, 1),
                        x_slice(c, dt, 1, TCH), op0=ALU.add, op1=ALU.mult)
                    nc.vector.scalar_tensor_tensor(
                        mbuf[:, 1024 + TCH:2048], p[3][:], b2s(dt, 3),
                        x_slice(c, dt, 3, TCH), op0=ALU.add, op1=ALU.mult)
                    # DVE: s-wide + fin for the previous iteration
                    emit_sw()
                    # ACT: silu + out DMA for the pair completed last iter
                    emit_silu()
                    if it % 2 == 0:
                        finbuf = wpool.tile([128, 1024], bf16, tag="fin")
                    sw_q.append((mbuf, finbuf, it % 2))
                    if it % 2 == 1:
                        silu_q.append((finbuf, it - 1))
                    if hps_next is not None and dt == d0 + 8:
                        mm1_silu(c + 1, hps_next)
            emit_sw()
            emit_silu(drain=True)
    nc.compile()
    return nc


def _prep_shards(x, w1, w2, b2, tok, d, h, halo):
    """Host-side shard prep. Returns list of per-core in_maps."""
    import ml_dtypes
    bf16 = ml_dtypes.bfloat16

    n_dt = d // 128
    n_hc = h // 128
    n_tc = tok // TCH
    b, t, _ = x.shape
    shards_per_batch = (b * t // tok) // b
    # w1 [D, H] -> [128, hc, dt, 128]: w1r[p, hc, dt, l] = w1[dt*128+p, hc*128+l]
    w1_r = np.ascontiguousarray(
        w1.reshape(n_dt, 128, n_hc, 128).transpose(1, 2, 0, 3)).astype(bf16)
    # w2 [h, d*W] -> [128, n_hc, n_dt, W, 128]
    w2_r = np.ascontiguousarray(
        w2.reshape(n_hc, 128, n_dt, 128, W)
        .transpose(1, 0, 2, 4, 3)).astype(bf16)
    b2_r = np.ascontiguousarray(
        b2.reshape(n_dt, 128, W).transpose(1, 0, 2)
        .reshape(128, n_dt * W)).astype(np.float32)

    in_maps = []
    for core in range(N_CORES):
        bi, half = divmod(core, shards_per_batch)
        t0 = half * tok
        xh = np.zeros((tok + halo, d), np.float32)
        lo = max(t0 - halo, 0)
        xh[halo - (t0 - lo):] = x[bi, lo: t0 + tok]
        xhT = xh.T.astype(bf16).reshape(n_dt, 128, tok + halo)  # [dt, p, col]
        # chunk-major with duplicated halo: col 0 of chunk c = token c*TCH-3
        xTc = np.zeros((128, n_tc, n_dt, XSTR), bf16)
        for c in range(n_tc):
            xTc[:, c, :, :TCH + halo] = (
                xhT[:, :, c * TCH: c * TCH + TCH + halo].transpose(1, 0, 2))
        in_maps.append({
            "xT": xTc, "w1": w1_r, "w2r": w2_r, "b2r": b2_r})
    return in_maps


_NC_CACHE = {}


def kernel(x, w1, w2, b2, trace=False):
    from concourse.bass_utils import run_bass_kernel_spmd

    tok, d, h = TOK, D, H
    key = (tok, d, h)
    if key not in _NC_CACHE:
        _NC_CACHE[key] = _build_nc(tok, d, h)
    nc = _NC_CACHE[key]

    in_maps = _prep_shards(
        np.asarray(x, np.float32), np.asarray(w1, np.float32),
        np.asarray(w2, np.float32), np.asarray(b2, np.float32),
        tok, d, h, HALO)

    res = run_bass_kernel_spmd(nc, in_maps, core_ids=list(range(N_CORES)),
                               trace=trace)
    kernel.last_result = res

    n_dt = d // 128
    n_tc = tok // TCH
    shards_per_batch = (B * T // tok) // B
    out = np.empty((B, T, D), np.float32)
    for core in range(N_CORES):
        bi, half = divmod(core, shards_per_batch)
        oT = res.results[core]["outT"]  # [128, n_tc*n_dt*TCH]
        # [128p, c, dt, j] -> [c, j, dt, p] -> [tok, d]
        o = oT.reshape(128, n_tc, n_dt, TCH).transpose(1, 3, 2, 0)
        out[bi, half * tok:(half + 1) * tok] = (
            o.reshape(tok, d).astype(np.float32))
    return out


# revision 19
# speedup vs baseline: 1.3891x; 1.3891x over previous
"""Trainium2 Bass kernel for nn_DynamicShortConvolution.

Reference computation (per token t, channel d):
    h    = silu(x @ w1)                       # [T, H]
    flat = h @ w2 + b2                        # [T, D*W]
    k    = flat.reshape(T, D, W)
    out[t, d] = silu(sum_w k[t, d, w] * x[t - (W-1) + w, d])

Sharding: 8 cores, each one (batch, half-of-T) shard of 2048 tokens plus a
3-token left halo.  All per-core tensors are laid out TRANSPOSED ([D, T],
channels on SBUF partitions) so the conv's token shift is a free-dim offset.

v5: the steady state (~2.7us per 512-token x 128-channel iteration) is at
its engine-balance optimum: DVE (2 stt + 2 adds) ~= GPSIMD (2 multiplies)
~= 2.7us, with ACT (2 psum pulls + silu) and PE (8+ matmuls) just under.
The remaining ~45us were lead-in and chunk-boundary overhead:
  - x is stored CHUNK-MAJOR in DRAM ([128, chunk, dt, 516], halo columns
    duplicated per chunk) so one chunk moves with 128 x 16.5KB descriptors
    instead of 2048 x 1KB - the first chunk lands ~6us earlier and DMA
    queues stay quiet during compute.
  - w1 is stored [128, hc, dt, 128] and sent as two DMAs; chunk 0 of x is
    sent as four quarter-DMAs so mm1's contraction steps chase arrival.
  - ~22 warmup matmuls on a memset scratch tile run while the input DMA
    is in flight, so mm1 executes at full clock (2.4GHz) instead of
    ramping from the 0.65GHz cold p-state.
  - mm1 for chunk c+1 is spread over iterations 4..11 of chunk c (silu at
    12) instead of 8..15 (silu at 15), removing the chunk-boundary stall.
"""

import numpy as np

# Problem constants (hardcoded per harness contract).
B, T, D, H, W = 4, 4096, 2048, 256, 4
HALO = W - 1
N_CORES = 8
TOK = (B * T) // N_CORES  # tokens per core = 2048
TCH = 512                 # token chunk
XSTR = TCH + HALO + 1     # per-(chunk,dtile) x columns (pad to even)


def _build_nc(tok, d, h, sim_safe=False):
    """Build the single-core Bass/Tile program."""
    import concourse.bass as bass
    import concourse.bacc as bacc
    import concourse.mybir as mybir
    import concourse.tile as tile

    f32 = mybir.dt.float32
    bf16 = mybir.dt.bfloat16
    AF = mybir.ActivationFunctionType
    ALU = mybir.AluOpType
    # CoreSim has no Silu; Sigmoid exercises the identical dataflow
    AF_ACT = AF.Sigmoid if sim_safe else AF.Silu

    n_dt = d // 128        # d tiles = 16
    n_hc = h // 128        # h tiles = 2
    n_tc = tok // TCH      # token chunks = 4

    nc = bacc.Bacc()

    # DRAM I/O. x chunk-major with duplicated halo: big contiguous runs.
    xT = nc.declare_dram_parameter("xT", [128, n_tc, n_dt, XSTR], bf16,
                                   isOutput=False)
    # w1r[p, hc, dt, hl] = w1[dt*128+p, hc*128+hl]
    w1 = nc.declare_dram_parameter("w1", [128, n_hc, n_dt, 128], bf16,
                                   isOutput=False)
    # w2r[hl, hc, dt, w, dl] = w2[hc*128+hl, ((dt*128+dl))*W + w]
    w2r = nc.declare_dram_parameter("w2r", [128, n_hc, n_dt, W, 128], bf16,
                                    isOutput=False)
    # b2r[p, dt*W + w] = b2[(dt*128+p)*W + w]
    b2r = nc.declare_dram_parameter("b2r", [128, n_dt * W], f32, isOutput=False)
    # outT[p, (c*n_dt + dt)*TCH + j] = out token c*TCH+j, channel dt*128+p
    outT = nc.declare_dram_parameter("outT", [128, n_tc * n_dt * TCH], bf16,
                                     isOutput=True)

    def strided_ap(base, pairs):
        """Rebuild an AP with explicit free-dim [stride, count] pairs."""
        new = [list(base.ap[0])] + [list(p) for p in pairs]
        return type(base)(base.tensor, base.offset, new)

    with tile.TileContext(nc) as tc:
        with (
            tc.tile_pool(name="resident", bufs=1) as rpool,
            tc.tile_pool(name="work", bufs=4) as wpool,
            tc.tile_pool(name="outstage", bufs=3) as opool,
            tc.tile_pool(name="psum2", bufs=6, space="PSUM") as ppool,
            tc.tile_pool(name="psumH", bufs=2, space="PSUM") as hpool,
        ):
            # ---- resident tiles ----
            xT_sb = rpool.tile([128, n_tc, n_dt, XSTR], bf16, tag="xT")
            w1_sb = rpool.tile([128, n_hc, n_dt, 128], bf16, tag="w1")
            w2_sb = rpool.tile([128, n_hc, n_dt, W, 128], bf16, tag="w2")
            b2_sb = rpool.tile([128, n_dt * W], f32, tag="b2")
            # hT chunk-major: [hc0 512 | hc1 512] per chunk
            hT_sb = rpool.tile([128, n_tc * 2 * TCH], bf16, tag="hT")
            scr = rpool.tile([128, TCH], bf16, tag="scr")   # warmup operand
            scr2 = rpool.tile([128, 64], bf16, tag="scr2")  # warmup drain

            # ---- PE warmup: ~22 dummy matmuls while input DMA flies ----
            nc.gpsimd.memset(scr[:], 0)
            for ww in range(11):
                wps = ppool.tile([128, TCH], f32, tag="ps", name="warm")
                nc.tensor.matmul(wps[:], scr[:, :128], scr[:],
                                 start=True, stop=False)
                nc.tensor.matmul(wps[:], scr[:, :128], scr[:],
                                 start=False, stop=True)
                nc.scalar.activation(scr2[:], wps[:, :64], AF.Identity)

            # ---- input DMA, lead-in ordered ----
            for hc in range(n_hc):
                nc.sync.dma_start(w1_sb[:, hc], w1[:, hc])
            for dt in range(0, n_dt, 4):  # chunk 0 in quarters
                nc.sync.dma_start(xT_sb[:, 0, dt:dt + 4], xT[:, 0, dt:dt + 4])
            for hc in range(n_hc):
                nc.sync.dma_start(w2_sb[:, hc, 0:4], w2r[:, hc, 0:4])
            nc.sync.dma_start(b2_sb[:], b2r[:])
            nc.sync.dma_start(xT_sb[:, 1], xT[:, 1])
            for hc in range(n_hc):
                nc.sync.dma_start(w2_sb[:, hc, 4:8], w2r[:, hc, 4:8])
            # x2, x3, w2[8:16] are dispatched from inside the loop so their
            # SBUF-write bursts don't collide with the first iterations
            # (input DMA traffic measured as the cause of early stalls)

            def x_slice(c, dt, col, n):
                # col 0 = token c*TCH - HALO (halo duplicated per chunk)
                return xT_sb[:, c, dt, col: col + n]

            def b2s(dt, w):
                return b2_sb[:, dt * W + w: dt * W + w + 1]

            def hslice(c, hc):
                return hT_sb[:, c * 1024 + hc * TCH: c * 1024 + (hc + 1) * TCH]

            def mm1_mms(c, hps, q):
                # contraction tile q of mm1 for chunk c
                for hc in range(n_hc):
                    nc.tensor.matmul(
                        hps[hc][:],
                        w1_sb[:, hc, q],
                        x_slice(c, q, HALO, TCH),
                        start=(q == 0), stop=(q == n_dt - 1),
                    )

            def mm1_silu(c, hps):
                for hc in range(n_hc):
                    nc.scalar.activation(hslice(c, hc), hps[hc][:], AF_ACT)

            def hps_alloc():
                return [hpool.tile([128, TCH], f32, tag="hps", name=f"hps{hc}")
                        for hc in range(n_hc)]

            # ---- chunk 0 mm1 up front (chases the quarter DMAs) ----
            hps = hps_alloc()
            for q in range(n_dt):
                mm1_mms(0, hps, q)
            mm1_silu(0, hps)

            sw_q = []    # (mbuf, finbuf, half): s-wide + fin, 1-iter skew
            silu_q = []  # (finbuf, git): completed pairs awaiting silu

            def emit_sw():
                while sw_q:
                    mb, fb, half = sw_q.pop(0)
                    sb = wpool.tile([128, 1024], bf16, tag="s")
                    nc.vector.tensor_tensor(
                        sb[:], mb[:, :1024], mb[:, 1024:], op=ALU.add)
                    nc.vector.tensor_tensor(
                        fb[:, half * TCH:(half + 1) * TCH],
                        sb[:, :TCH], sb[:, TCH:], op=ALU.add)

            otbuf = [None]

            def emit_silu(drain=False):
                # one [128,4096] output DMA per four silu pairs
                while silu_q:
                    fb, git = silu_q.pop(0)
                    part = (git % 8) // 2
                    if part == 0:
                        otbuf[0] = opool.tile([128, 4096], bf16, tag="ot",
                                              name="ot")
                    ot = otbuf[0]
                    nc.scalar.activation(
                        ot[:, part * 1024:(part + 1) * 1024], fb[:], AF_ACT)
                    last_group = git >= n_tc * n_dt - 8
                    if last_group and part == 1:
                        # final group: ship the first half early so the
                        # last DMA overlaps the remaining tail silus
                        nc.sync.dma_start(
                            outT[:, (git - 2) * TCH: git * TCH + 1024],
                            ot[:, :2048])
                    elif last_group and part == 3:
                        nc.sync.dma_start(
                            outT[:, (git - 2) * TCH: git * TCH + 1024],
                            ot[:, 2048:4096])
                    elif part == 3 or drain:
                        nc.sync.dma_start(
                            outT[:, (git - 2 * part) * TCH:
                                 git * TCH + 1024], ot[:, :(part + 1) * 1024])

            finbuf = None
            for c in range(n_tc):
                hps_next = hps_alloc() if c + 1 < n_tc else None
                for dt in range(n_dt):
                    it = c * n_dt + dt
                    # deferred input DMA dispatches (Sync issues in order),
                    # split small so each SBUF-write burst is short
                    if it in (3, 6):
                        a = 8 if it == 3 else 12
                        for hc in range(n_hc):
                            nc.sync.dma_start(w2_sb[:, hc, a:a + 4],
                                              w2r[:, hc, a:a + 4])
                    elif 10 <= it <= 16 and it % 2 == 0:
                        q4 = 2 * (it - 10)  # 0,4,8,12
                        nc.sync.dma_start(xT_sb[:, 2, q4:q4 + 4],
                                          xT[:, 2, q4:q4 + 4])
                    elif 22 <= it <= 28 and it % 2 == 0:
                        q4 = 2 * (it - 22)
                        nc.sync.dma_start(xT_sb[:, 3, q4:q4 + 4],
                                          xT[:, 3, q4:q4 + 4])
                    # PE: per-tap psum tiles, ACT-feeding taps (0,2) first
                    p = [None] * W
                    for w in (0, 2, 1, 3):
                        pt = ppool.tile([128, TCH], f32, tag="ps")
                        p[w] = pt
                        for hc in range(n_hc):
                            nc.tensor.matmul(
                                pt[:], w2_sb[:, hc, dt, w], hslice(c, hc),
                                start=(hc == 0), stop=(hc == n_hc - 1),
                            )
                    # PE: two contraction tiles of mm1(c+1), spread over
                    # iters 4..11 (x for c+1 lands early with chunk-major
                    # DMA); silu at iter 12 so hT(c+1) is ready with slack.
                    # Chunk 0 spreads over 6..13 instead - its early iters
                    # are still filling the psum pipeline.
                    d0 = 6 if c == 0 else 4
                    if hps_next is not None and d0 <= dt < d0 + 8:
                        q = 2 * (dt - d0)
                        mm1_mms(c + 1, hps_next, q)
                        mm1_mms(c + 1, hps_next, q + 1)
                    # ACT: pull taps 0,2 to bf16 with the b2 bias fused
                    t02 = wpool.tile([128, 1024], bf16, tag="t02")
                    nc.scalar.activation(t02[:, :TCH], p[0][:],
                                         AF.Identity, bias=b2s(dt, 0))
                    nc.scalar.activation(t02[:, TCH:], p[2][:],
                                         AF.Identity, bias=b2s(dt, 2))
                    # GPSIMD: multiply pulled taps by their x windows
                    # (no stt opcode on GPSIMD - fails at NEFF compile;
                    # moving these to DVE or the s-wide add to GPSIMD both
                    # measured ~40% slower end-to-end)
                    mbuf = wpool.tile([128, 2048], bf16, tag="m")
                    nc.gpsimd.tensor_tensor(
                        mbuf[:, 0:TCH], t02[:, :TCH],
                        x_slice(c, dt, 0, TCH), op=ALU.mult)
                    nc.gpsimd.tensor_tensor(
                        mbuf[:, 1024:1024 + TCH], t02[:, TCH:],
                        x_slice(c, dt, 2, TCH), op=ALU.mult)
                    # DVE: f32 stt (bias + x-mult) for taps 1,3
                    nc.vector.scalar_tensor_tensor(
                        mbuf[:, TCH:1024], p[1][:], b2s(dt, 1),
                        x_slice(c, dt, 1, TCH), op0=ALU.add, op1=ALU.mult)
                    nc.vector.scalar_tensor_tensor(
                        mbuf[:, 1024 + TCH:2048], p[3][:], b2s(dt, 3),
                        x_slice(c, dt, 3, TCH), op0=ALU.add, op1=ALU.mult)
                    # DVE: s-wide + fin for the previous iteration
                    emit_sw()
                    # ACT: silu + out DMA for the pair completed last iter
                    emit_silu()
                    if it % 2 == 0:
                        finbuf = wpool.tile([128, 1024], bf16, tag="fin")
                    sw_q.append((mbuf, finbuf, it % 2))
                    if it % 2 == 1:
                        silu_q.append((finbuf, it - 1))
                    if hps_next is not None and dt == d0 + 8:
                        mm1_silu(c + 1, hps_next)
            emit_sw()
            emit_silu(drain=True)
    nc.compile()
    return nc


def _prep_shards(x, w1, w2, b2, tok, d, h, halo):
    """Host-side shard prep. Returns list of per-core in_maps."""
    import ml_dtypes
    bf16 = ml_dtypes.bfloat16

    n_dt = d // 128
    n_hc = h // 128
    n_tc = tok // TCH
    b, t, _ = x.shape
    shards_per_batch = (b * t // tok) // b
    # w1 [D, H] -> [128, hc, dt, 128]: w1r[p, hc, dt, l] = w1[dt*128+p, hc*128+l]
    w1_r = np.ascontiguousarray(
        w1.reshape(n_dt, 128, n_hc, 128).transpose(1, 2, 0, 3)).astype(bf16)
    # w2 [h, d*W] -> [128, n_hc, n_dt, W, 128]
    w2_r = np.ascontiguousarray(
        w2.reshape(n_hc, 128, n_dt, 128, W)
        .transpose(1, 0, 2, 4, 3)).astype(bf16)
    b2_r = np.ascontiguousarray(
        b2.reshape(n_dt, 128, W).transpose(1, 0, 2)
        .reshape(128, n_dt * W)).astype(np.float32)

    in_maps = []
    for core in range(N_CORES):
        bi, half = divmod(core, shards_per_batch)
        t0 = half * tok
        xh = np.zeros((tok + halo, d), np.float32)
        lo = max(t0 - halo, 0)
        xh[halo - (t0 - lo):] = x[bi, lo: t0 + tok]
        xhT = xh.T.astype(bf16).reshape(n_dt, 128, tok + halo)  # [dt, p, col]
        # chunk-major with duplicated halo: col 0 of chunk c = token c*TCH-3
        xTc = np.zeros((128, n_tc, n_dt, XSTR), bf16)
        for c in range(n_tc):
            xTc[:, c, :, :TCH + halo] = (
                xhT[:, :, c * TCH: c * TCH + TCH + halo].transpose(1, 0, 2))
        in_maps.append({
            "xT": xTc, "w1": w1_r, "w2r": w2_r, "b2r": b2_r})
    return in_maps


_NC_CACHE = {}


def kernel(x, w1, w2, b2, trace=False):
    from concourse.bass_utils import run_bass_kernel_spmd

    tok, d, h = TOK, D, H
    key = (tok, d, h)
    if key not in _NC_CACHE:
        _NC_CACHE[key] = _build_nc(tok, d, h)
    nc = _NC_CACHE[key]

    in_maps = _prep_shards(
        np.asarray(x, np.float32), np.asarray(w1, np.float32),
        np.asarray(w2, np.float32), np.asarray(b2, np.float32),
        tok, d, h, HALO)

    res = run_bass_kernel_spmd(nc, in_maps, core_ids=list(range(N_CORES)),
                               trace=trace)
    kernel.last_result = res

    n_dt = d // 128
    n_tc = tok // TCH
    shards_per_batch = (B * T // tok) // B
    out = np.empty((B, T, D), np.float32)
    for core in range(N_CORES):
        bi, half = divmod(core, shards_per_batch)
        oT = res.results[core]["outT"]  # [128, n_tc*n_dt*TCH]
        # [128p, c, dt, j] -> [c, j, dt, p] -> [tok, d]
        o = oT.reshape(128, n_tc, n_dt, TCH).transpose(1, 3, 2, 0)
        out[bi, half * tok:(half + 1) * tok] = (
            o.reshape(tok, d).astype(np.float32))
    return out


# revision 21
# speedup vs baseline: 1.4268x; 1.0271x over previous
"""Trainium2 Bass kernel for nn_DynamicShortConvolution.

Reference computation (per token t, channel d):
    h    = silu(x @ w1)                       # [T, H]
    flat = h @ w2 + b2                        # [T, D*W]
    k    = flat.reshape(T, D, W)
    out[t, d] = silu(sum_w k[t, d, w] * x[t - (W-1) + w, d])

Sharding: 8 cores, each one (batch, half-of-T) shard of 2048 tokens plus a
3-token left halo.  All per-core tensors are laid out TRANSPOSED ([D, T],
channels on SBUF partitions) so the conv's token shift is a free-dim offset.

v5: the steady state (~2.7us per 512-token x 128-channel iteration) is at
its engine-balance optimum: DVE (2 stt + 2 adds) ~= GPSIMD (2 multiplies)
~= 2.7us, with ACT (2 psum pulls + silu) and PE (8+ matmuls) just under.
The remaining ~45us were lead-in and chunk-boundary overhead:
  - x is stored CHUNK-MAJOR in DRAM ([128, chunk, dt, 516], halo columns
    duplicated per chunk) so one chunk moves with 128 x 16.5KB descriptors
    instead of 2048 x 1KB - the first chunk lands ~6us earlier and DMA
    queues stay quiet during compute.
  - w1 is stored [128, hc, dt, 128] and sent as two DMAs; chunk 0 of x is
    sent as four quarter-DMAs so mm1's contraction steps chase arrival.
  - ~22 warmup matmuls on a memset scratch tile run while the input DMA
    is in flight, so mm1 executes at full clock (2.4GHz) instead of
    ramping from the 0.65GHz cold p-state.
  - mm1 for chunk c+1 is spread over iterations 4..11 of chunk c (silu at
    12) instead of 8..15 (silu at 15), removing the chunk-boundary stall.
"""

import numpy as np

# Problem constants (hardcoded per harness contract).
B, T, D, H, W = 4, 4096, 2048, 256, 4
HALO = W - 1
N_CORES = 8
TOK = (B * T) // N_CORES  # tokens per core = 2048
TCH = 512                 # token chunk
XSTR = TCH + HALO + 1     # per-(chunk,dtile) x columns (pad to even)


def _build_nc(tok, d, h, sim_safe=False):
    """Build the single-core Bass/Tile program."""
    import concourse.bass as bass
    import concourse.bacc as bacc
    import concourse.mybir as mybir
    import concourse.tile as tile

    f32 = mybir.dt.float32
    bf16 = mybir.dt.bfloat16
    AF = mybir.ActivationFunctionType
    ALU = mybir.AluOpType
    # CoreSim has no Silu; Sigmoid exercises the identical dataflow
    AF_ACT = AF.Sigmoid if sim_safe else AF.Silu

    n_dt = d // 128        # d tiles = 16
    n_hc = h // 128        # h tiles = 2
    n_tc = tok // TCH      # token chunks = 4

    nc = bacc.Bacc()

    # DRAM I/O. x chunk-major with duplicated halo: big contiguous runs.
    xT = nc.declare_dram_parameter("xT", [128, n_tc, n_dt, XSTR], bf16,
                                   isOutput=False)
    # w1r[p, hc, dt, hl] = w1[dt*128+p, hc*128+hl]
    w1 = nc.declare_dram_parameter("w1", [128, n_hc, n_dt, 128], bf16,
                                   isOutput=False)
    # w2r[hl, hc, dt, w, dl] = w2[hc*128+hl, ((dt*128+dl))*W + w]
    w2r = nc.declare_dram_parameter("w2r", [128, n_hc, n_dt, W, 128], bf16,
                                    isOutput=False)
    # b2r[p, dt*W + w] = b2[(dt*128+p)*W + w]
    b2r = nc.declare_dram_parameter("b2r", [128, n_dt * W], f32, isOutput=False)
    # outT[p, (c*n_dt + dt)*TCH + j] = out token c*TCH+j, channel dt*128+p
    outT = nc.declare_dram_parameter("outT", [128, n_tc * n_dt * TCH], bf16,
                                     isOutput=True)

    def strided_ap(base, pairs):
        """Rebuild an AP with explicit free-dim [stride, count] pairs."""
        new = [list(base.ap[0])] + [list(p) for p in pairs]
        return type(base)(base.tensor, base.offset, new)

    with tile.TileContext(nc) as tc:
        with (
            tc.tile_pool(name="resident", bufs=1) as rpool,
            tc.tile_pool(name="work", bufs=4) as wpool,
            tc.tile_pool(name="outstage", bufs=3) as opool,
            tc.tile_pool(name="psum2", bufs=6, space="PSUM") as ppool,
            tc.tile_pool(name="psumH", bufs=2, space="PSUM") as hpool,
        ):
            # ---- resident tiles ----
            xT_sb = rpool.tile([128, n_tc, n_dt, XSTR], bf16, tag="xT")
            w1_sb = rpool.tile([128, n_hc, n_dt, 128], bf16, tag="w1")
            w2_sb = rpool.tile([128, n_hc, n_dt, W, 128], bf16, tag="w2")
            b2_sb = rpool.tile([128, n_dt * W], f32, tag="b2")
            # hT chunk-major: [hc0 512 | hc1 512] per chunk
            hT_sb = rpool.tile([128, n_tc * 2 * TCH], bf16, tag="hT")
            scr = rpool.tile([128, TCH], bf16, tag="scr")   # warmup operand
            scr2 = rpool.tile([128, 64], bf16, tag="scr2")  # warmup drain

            # ---- PE warmup: ~22 dummy matmuls while input DMA flies ----
            nc.gpsimd.memset(scr[:], 0)
            for ww in range(11):
                wps = ppool.tile([128, TCH], f32, tag="ps", name="warm")
                nc.tensor.matmul(wps[:], scr[:, :128], scr[:],
                                 start=True, stop=False)
                nc.tensor.matmul(wps[:], scr[:, :128], scr[:],
                                 start=False, stop=True)
                nc.scalar.activation(scr2[:], wps[:, :64], AF.Identity)
            # preload the Silu activation table off the critical path
            nc.scalar.activation(scr2[:], scr[:, :64], AF_ACT)

            # ---- input DMA, lead-in ordered ----
            for hc in range(n_hc):
                nc.sync.dma_start(w1_sb[:, hc], w1[:, hc])
            for dt in range(0, n_dt, 4):  # chunk 0 in quarters
                nc.sync.dma_start(xT_sb[:, 0, dt:dt + 4], xT[:, 0, dt:dt + 4])
            for hc in range(n_hc):
                nc.sync.dma_start(w2_sb[:, hc, 0:4], w2r[:, hc, 0:4])
            nc.sync.dma_start(b2_sb[:], b2r[:])
            nc.sync.dma_start(xT_sb[:, 1], xT[:, 1])
            for hc in range(n_hc):
                nc.sync.dma_start(w2_sb[:, hc, 4:8], w2r[:, hc, 4:8])
            # x2, x3, w2[8:16] are dispatched from inside the loop so their
            # SBUF-write bursts don't collide with the first iterations
            # (input DMA traffic measured as the cause of early stalls)

            def x_slice(c, dt, col, n):
                # col 0 = token c*TCH - HALO (halo duplicated per chunk)
                return xT_sb[:, c, dt, col: col + n]

            def b2s(dt, w):
                return b2_sb[:, dt * W + w: dt * W + w + 1]

            def hslice(c, hc):
                return hT_sb[:, c * 1024 + hc * TCH: c * 1024 + (hc + 1) * TCH]

            def mm1_mms(c, hps, q):
                # contraction tile q of mm1 for chunk c
                for hc in range(n_hc):
                    nc.tensor.matmul(
                        hps[hc][:],
                        w1_sb[:, hc, q],
                        x_slice(c, q, HALO, TCH),
                        start=(q == 0), stop=(q == n_dt - 1),
                    )

            def mm1_silu(c, hps):
                for hc in range(n_hc):
                    nc.scalar.activation(hslice(c, hc), hps[hc][:], AF_ACT)

            def hps_alloc():
                return [hpool.tile([128, TCH], f32, tag="hps", name=f"hps{hc}")
                        for hc in range(n_hc)]

            # ---- chunk 0 mm1 up front (chases the quarter DMAs) ----
            hps = hps_alloc()
            for q in range(n_dt):
                mm1_mms(0, hps, q)
            mm1_silu(0, hps)

            sw_q = []    # (mbuf, finbuf, half): s-wide + fin, 1-iter skew
            silu_q = []  # (finbuf, git): completed pairs awaiting silu

            def emit_sw():
                while sw_q:
                    mb, fb, half = sw_q.pop(0)
                    sb = wpool.tile([128, 1024], bf16, tag="s")
                    nc.vector.tensor_tensor(
                        sb[:], mb[:, :1024], mb[:, 1024:], op=ALU.add)
                    nc.vector.tensor_tensor(
                        fb[:, half * TCH:(half + 1) * TCH],
                        sb[:, :TCH], sb[:, TCH:], op=ALU.add)

            otbuf = [None]

            def emit_silu(drain=False):
                # one [128,4096] output DMA per four silu pairs
                while silu_q:
                    fb, git = silu_q.pop(0)
                    part = (git % 8) // 2
                    if part == 0:
                        otbuf[0] = opool.tile([128, 4096], bf16, tag="ot",
                                              name="ot")
                    ot = otbuf[0]
                    nc.scalar.activation(
                        ot[:, part * 1024:(part + 1) * 1024], fb[:], AF_ACT)
                    last_group = git >= n_tc * n_dt - 8
                    if last_group and part == 1:
                        # final group: ship the first half early so the
                        # last DMA overlaps the remaining tail silus
                        nc.sync.dma_start(
                            outT[:, (git - 2) * TCH: git * TCH + 1024],
                            ot[:, :2048])
                    elif last_group and part == 3:
                        nc.sync.dma_start(
                            outT[:, (git - 2) * TCH: git * TCH + 1024],
                            ot[:, 2048:4096])
                    elif part == 3 or drain:
                        nc.sync.dma_start(
                            outT[:, (git - 2 * part) * TCH:
                                 git * TCH + 1024], ot[:, :(part + 1) * 1024])

            finbuf = None
            for c in range(n_tc):
                hps_next = hps_alloc() if c + 1 < n_tc else None
                for dt in range(n_dt):
                    it = c * n_dt + dt
                    # deferred input DMA dispatches (Sync issues in order),
                    # split small so each SBUF-write burst is short
                    if it in (3, 6):
                        a = 8 if it == 3 else 12
                        for hc in range(n_hc):
                            nc.sync.dma_start(w2_sb[:, hc, a:a + 4],
                                              w2r[:, hc, a:a + 4])
                    elif 10 <= it <= 16 and it % 2 == 0:
                        q4 = 2 * (it - 10)  # 0,4,8,12
                        nc.sync.dma_start(xT_sb[:, 2, q4:q4 + 4],
                                          xT[:, 2, q4:q4 + 4])
                    elif 22 <= it <= 28 and it % 2 == 0:
                        q4 = 2 * (it - 22)
                        nc.sync.dma_start(xT_sb[:, 3, q4:q4 + 4],
                                          xT[:, 3, q4:q4 + 4])
                    # PE: per-tap psum tiles, ACT-feeding taps (0,2) first
                    p = [None] * W
                    for w in (0, 2, 1, 3):
                        pt = ppool.tile([128, TCH], f32, tag="ps")
                        p[w] = pt
                        for hc in range(n_hc):
                            nc.tensor.matmul(
                                pt[:], w2_sb[:, hc, dt, w], hslice(c, hc),
                                start=(hc == 0), stop=(hc == n_hc - 1),
                            )
                    # PE: two contraction tiles of mm1(c+1), spread over
                    # iters 4..11 (x for c+1 lands early with chunk-major
                    # DMA); silu at iter 12 so hT(c+1) is ready with slack.
                    # Chunk 0 spreads over 6..13 instead - its early iters
                    # are still filling the psum pipeline.
                    d0 = 6 if c == 0 else 4
                    if hps_next is not None and d0 <= dt < d0 + 8:
                        q = 2 * (dt - d0)
                        mm1_mms(c + 1, hps_next, q)
                        mm1_mms(c + 1, hps_next, q + 1)
                    mbuf = wpool.tile([128, 2048], bf16, tag="m")
                    if it < 2:
                        # pipeline fill: drain taps 0,2 with DVE stt too -
                        # psum slots recycle at stt latency instead of the
                        # longer ACT-pull -> GPSIMD-mult chain latency
                        nc.vector.scalar_tensor_tensor(
                            mbuf[:, 0:TCH], p[0][:], b2s(dt, 0),
                            x_slice(c, dt, 0, TCH), op0=ALU.add,
                            op1=ALU.mult)
                        nc.vector.scalar_tensor_tensor(
                            mbuf[:, 1024:1024 + TCH], p[2][:], b2s(dt, 2),
                            x_slice(c, dt, 2, TCH), op0=ALU.add,
                            op1=ALU.mult)
                    else:
                        # ACT: pull taps 0,2 to bf16 with the b2 bias fused
                        t02 = wpool.tile([128, 1024], bf16, tag="t02")
                        nc.scalar.activation(t02[:, :TCH], p[0][:],
                                             AF.Identity, bias=b2s(dt, 0))
                        nc.scalar.activation(t02[:, TCH:], p[2][:],
                                             AF.Identity, bias=b2s(dt, 2))
                        # GPSIMD: multiply pulled taps by their x windows
                        # (no stt opcode on GPSIMD - fails at NEFF compile;
                        # moving these to DVE, or the s-wide add to GPSIMD,
                        # both measured ~40% slower end-to-end)
                        nc.gpsimd.tensor_tensor(
                            mbuf[:, 0:TCH], t02[:, :TCH],
                            x_slice(c, dt, 0, TCH), op=ALU.mult)
                        nc.gpsimd.tensor_tensor(
                            mbuf[:, 1024:1024 + TCH], t02[:, TCH:],
                            x_slice(c, dt, 2, TCH), op=ALU.mult)
                    # DVE: f32 stt (bias + x-mult) for taps 1,3
                    nc.vector.scalar_tensor_tensor(
                        mbuf[:, TCH:1024], p[1][:], b2s(dt, 1),
                        x_slice(c, dt, 1, TCH), op0=ALU.add, op1=ALU.mult)
                    nc.vector.scalar_tensor_tensor(
                        mbuf[:, 1024 + TCH:2048], p[3][:], b2s(dt, 3),
                        x_slice(c, dt, 3, TCH), op0=ALU.add, op1=ALU.mult)
                    # DVE: s-wide + fin for the previous iteration
                    emit_sw()
                    # ACT: silu + out DMA for the pair completed last iter
                    emit_silu()
                    if it % 2 == 0:
                        finbuf = wpool.tile([128, 1024], bf16, tag="fin")
                    sw_q.append((mbuf, finbuf, it % 2))
                    if it % 2 == 1:
                        silu_q.append((finbuf, it - 1))
                    if hps_next is not None and dt == d0 + 8:
                        mm1_silu(c + 1, hps_next)
            emit_sw()
            emit_silu(drain=True)
    nc.compile()
    return nc


def _prep_shards(x, w1, w2, b2, tok, d, h, halo):
    """Host-side shard prep. Returns list of per-core in_maps."""
    import ml_dtypes
    bf16 = ml_dtypes.bfloat16

    n_dt = d // 128
    n_hc = h // 128
    n_tc = tok // TCH
    b, t, _ = x.shape
    shards_per_batch = (b * t // tok) // b
    # w1 [D, H] -> [128, hc, dt, 128]: w1r[p, hc, dt, l] = w1[dt*128+p, hc*128+l]
    w1_r = np.ascontiguousarray(
        w1.reshape(n_dt, 128, n_hc, 128).transpose(1, 2, 0, 3)).astype(bf16)
    # w2 [h, d*W] -> [128, n_hc, n_dt, W, 128]
    w2_r = np.ascontiguousarray(
        w2.reshape(n_hc, 128, n_dt, 128, W)
        .transpose(1, 0, 2, 4, 3)).astype(bf16)
    b2_r = np.ascontiguousarray(
        b2.reshape(n_dt, 128, W).transpose(1, 0, 2)
        .reshape(128, n_dt * W)).astype(np.float32)

    in_maps = []
    for core in range(N_CORES):
        bi, half = divmod(core, shards_per_batch)
        t0 = half * tok
        xh = np.zeros((tok + halo, d), np.float32)
        lo = max(t0 - halo, 0)
        xh[halo - (t0 - lo):] = x[bi, lo: t0 + tok]
        xhT = xh.T.astype(bf16).reshape(n_dt, 128, tok + halo)  # [dt, p, col]
        # chunk-major with duplicated halo: col 0 of chunk c = token c*TCH-3
        xTc = np.zeros((128, n_tc, n_dt, XSTR), bf16)
        for c in range(n_tc):
            xTc[:, c, :, :TCH + halo] = (
                xhT[:, :, c * TCH: c * TCH + TCH + halo].transpose(1, 0, 2))
        in_maps.append({
            "xT": xTc, "w1": w1_r, "w2r": w2_r, "b2r": b2_r})
    return in_maps


_NC_CACHE = {}


def kernel(x, w1, w2, b2, trace=False):
    from concourse.bass_utils import run_bass_kernel_spmd

    tok, d, h = TOK, D, H
    key = (tok, d, h)
    if key not in _NC_CACHE:
        _NC_CACHE[key] = _build_nc(tok, d, h)
    nc = _NC_CACHE[key]

    in_maps = _prep_shards(
        np.asarray(x, np.float32), np.asarray(w1, np.float32),
        np.asarray(w2, np.float32), np.asarray(b2, np.float32),
        tok, d, h, HALO)

    res = run_bass_kernel_spmd(nc, in_maps, core_ids=list(range(N_CORES)),
                               trace=trace)
    kernel.last_result = res

    n_dt = d // 128
    n_tc = tok // TCH
    shards_per_batch = (B * T // tok) // B
    out = np.empty((B, T, D), np.float32)
    for core in range(N_CORES):
        bi, half = divmod(core, shards_per_batch)
        oT = res.results[core]["outT"]  # [128, n_tc*n_dt*TCH]
        # [128p, c, dt, j] -> [c, j, dt, p] -> [tok, d]
        o = oT.reshape(128, n_tc, n_dt, TCH).transpose(1, 3, 2, 0)
        out[bi, half * tok:(half + 1) * tok] = (
            o.reshape(tok, d).astype(np.float32))
    return out
